# revision 6
# baseline (speedup 1.0000x reference)
"""Bahdanau additive attention (nn_AttentionLayer_72353019068602) on 8 trn2 cores.

Sharding: data-parallel over batch (32 -> 4 per core); W_a/U_a/V_a replicated.

Per-core pipeline (4 batches, software-pipelined so PE never idles):
  prefetch(b): enc[b] HBM->SBUF, cast bf16 (DVE), transpose (XBAR DMA)
  q[f,b]   = U_a.T @ dec_last.T          (PE, fp32, once)
  S[f,t]   = W_a.T @ encT                (PE, bf16, fp32 PSUM accum)
  th[f,t]  = tanh(S + q[f,b])            (ACT, per-partition bias)
  sc[1,t]  = V.T @ th                    (PE, M=1)
  softmax over t=2048                    (DVE+ACT on one partition)
  ctx[e]   = sum_t w[t]*encT[e,t]        (DVE mult+reduce, bf16)

prefetch(b+1) is issued BEFORE pass1(b) so each engine's FIFO has next-batch
work queued ahead of this batch's softmax/context chain -> no PE gap at batch
boundaries (HAM stays at K=8/8).
"""

import numpy as np

import concourse.bass as bass
import concourse.mybir as mybir
import concourse.tile as tile
from concourse import bacc
from concourse.bass_utils import run_bass_kernel_spmd

B, TE, TD, D = 32, 2048, 64, 1024
N_CORES = 8
BPC = B // N_CORES  # batches per core

F32 = mybir.dt.float32
BF16 = mybir.dt.bfloat16

# test.py can flip this to get an NTFF profile / exec_time_ns out of the run.
TRACE = False
LAST_RESULTS = None  # BassKernelResults of the most recent kernel() call


def build_program():
    nc = bacc.Bacc("TRN2", target_bir_lowering=False, debug=False)

    enc = nc.dram_tensor("enc", [BPC, TE, D], F32, kind="ExternalInput").ap()
    decT = nc.dram_tensor("decT", [D, BPC], F32, kind="ExternalInput").ap()
    W = nc.dram_tensor("W", [D, D], F32, kind="ExternalInput").ap()
    U = nc.dram_tensor("U", [D, D], F32, kind="ExternalInput").ap()
    V = nc.dram_tensor("V", [D, 1], F32, kind="ExternalInput").ap()
    ctx_out = nc.dram_tensor("ctx_out", [BPC, D], F32, kind="ExternalOutput").ap()
    w_out = nc.dram_tensor("w_out", [BPC, TE], F32, kind="ExternalOutput").ap()

    EC = D // 128  # 8 e (and f) chunks
    NS = TE // 128  # 16 natural subtiles per batch
    NQ = TE // 512  # 4 t-chunks of 512 per batch
    Tanh = mybir.ActivationFunctionType.Tanh
    Exp = mybir.ActivationFunctionType.Exp

    with tile.TileContext(nc) as tc:
        with (
            tc.tile_pool(name="const", bufs=1) as const_pool,
            tc.tile_pool(name="nat", bufs=4) as nat_pool,
            tc.tile_pool(name="natb", bufs=4) as natb_pool,
            tc.tile_pool(name="encT", bufs=2) as encT_pool,
            tc.tile_pool(name="th", bufs=4) as th_pool,
            tc.tile_pool(name="sm", bufs=2) as sm_pool,
            tc.tile_pool(name="wrep", bufs=2) as wrep_pool,
            tc.tile_pool(name="junk", bufs=2) as junk_pool,
            tc.tile_pool(name="mmps", bufs=4, space="PSUM") as mm_psum,
            tc.tile_pool(name="scps", bufs=2, space="PSUM") as sc_psum,
            tc.tile_pool(name="qps", bufs=2, space="PSUM") as q_psum,
        ):
            encT_tiles = {}

            def prefetch(b):
                # encT[p, s, o, t]: e = o*128+p, global t = s*128+t
                encT = encT_pool.tile(
                    [128, NS, EC, 128], BF16, tag="encT", name="encT"
                )
                encT_tiles[b] = encT
                for s in range(NS):
                    nat = nat_pool.tile([128, D], F32, tag="nat", name="nat")
                    nc.sync.dma_start(nat, enc[b, s * 128 : (s + 1) * 128, :])
                    natb = natb_pool.tile([128, D], BF16, tag="natb", name="natb")
                    nc.vector.tensor_copy(natb, nat)
                    nc.sync.dma_start_transpose(encT[:, s], natb)

            # batch 0 prefetch goes first so its transposes clear the queues
            # while the weight preload + q matmuls run.
            prefetch(0)

            # ---- preload: W (cast bf16), V (cast bf16), dec_last.T, q = U.T @ decT
            W_bf = const_pool.tile([128, EC, D], BF16)
            for ec in range(EC):
                wt = nat_pool.tile([128, D], F32, tag="nat", name="wt")
                nc.sync.dma_start(wt, W[ec * 128 : (ec + 1) * 128, :])
                nc.vector.tensor_copy(W_bf[:, ec, :], wt)

            vt = const_pool.tile([128, EC], F32)
            nc.sync.dma_start(vt, V[:, 0].rearrange("(o p) -> p o", p=128))
            V_bf = const_pool.tile([128, EC], BF16)
            nc.vector.tensor_copy(V_bf, vt)

            dec_sb = const_pool.tile([128, EC, BPC], F32)
            nc.sync.dma_start(dec_sb, decT.rearrange("(o p) b -> p o b", p=128))

            q_sb = const_pool.tile([128, EC, BPC], F32)
            for fc in range(EC):
                qp = q_psum.tile([128, BPC], F32, tag="qp", name="qp")
                for dc in range(EC):
                    ut = nat_pool.tile([128, 128], F32, tag="ut", name="ut")
                    nc.sync.dma_start(
                        ut, U[dc * 128 : (dc + 1) * 128, fc * 128 : (fc + 1) * 128]
                    )
                    nc.tensor.matmul(
                        qp,
                        lhsT=ut,
                        rhs=dec_sb[:, dc, :],
                        start=(dc == 0),
                        stop=(dc == EC - 1),
                    )
                nc.scalar.copy(q_sb[:, fc, :], qp)

            # ---- main loop over batches
            for b in range(BPC):
                if b + 1 < BPC:
                    prefetch(b + 1)
                encT = encT_tiles.pop(b)

                scores = sm_pool.tile([1, TE], F32, tag="scores", name="scores")
                for tq in range(NQ):
                    scp = sc_psum.tile([1, 512], F32, tag="scp", name="scp")
                    rhs = encT[:, tq * 4 : (tq + 1) * 4]  # [128, 4, EC, 128]
                    for fc in range(EC):
                        ps = mm_psum.tile([128, 512], F32, tag="ps", name="ps")
                        for ec in range(EC):
                            nc.tensor.matmul(
                                ps,
                                lhsT=W_bf[:, ec, fc * 128 : (fc + 1) * 128],
                                rhs=rhs[:, :, ec, :],  # [128, 4, 128] = 512 cols
                                start=(ec == 0),
                                stop=(ec == EC - 1),
                            )
                        th = th_pool.tile([128, 512], BF16, tag="th", name="th")
                        nc.scalar.activation(th, ps, Tanh, bias=q_sb[:, fc, b : b + 1])
                        nc.tensor.matmul(
                            scp,
                            lhsT=V_bf[:, fc : fc + 1],
                            rhs=th,
                            start=(fc == 0),
                            stop=(fc == EC - 1),
                        )
                    nc.vector.tensor_copy(scores[:, tq * 512 : (tq + 1) * 512], scp)

                # softmax over the 2048 scores (single partition, in place)
                negmx = sm_pool.tile([1, 1], F32, tag="negmx", name="negmx")
                nc.vector.tensor_reduce(
                    negmx, scores, axis=mybir.AxisListType.X,
                    op=mybir.AluOpType.max, negate=True,
                )
                ssum = sm_pool.tile([1, 1], F32, tag="ssum", name="ssum")
                nc.scalar.activation(scores, scores, Exp, bias=negmx, accum_out=ssum)
                rec = sm_pool.tile([1, 1], F32, tag="rec", name="rec")
                nc.vector.reciprocal(rec, ssum)
                nc.vector.tensor_scalar_mul(scores, scores, rec)
                nc.sync.dma_start(w_out[b], scores)

                # context: ctx[e] = sum_t w[t] * encT[e, t]  (bf16 on DVE)
                scores_bf = sm_pool.tile([1, TE], BF16, tag="scores_bf", name="scores_bf")
                nc.vector.tensor_copy(scores_bf, scores)
                w_rep = wrep_pool.tile([128, TE], BF16, tag="wrep", name="w_rep")
                nc.gpsimd.partition_broadcast(w_rep, scores_bf)
                w_rep3 = w_rep.rearrange("p (s t) -> p s t", t=128)
                ctx_sb = sm_pool.tile([128, EC], F32, tag="ctxsb", name="ctx_sb")
                for ec in range(EC):
                    junk = junk_pool.tile([128, TE], BF16, tag="junk", name="junk")
                    nc.vector.tensor_tensor(
                        junk.rearrange("p (s t) -> p s t", t=128),
                        encT[:, :, ec, :],
                        w_rep3,
                        mybir.AluOpType.mult,
                    )
                    nc.vector.tensor_reduce(
                        ctx_sb[:, ec : ec + 1], junk,
                        axis=mybir.AxisListType.X, op=mybir.AluOpType.add,
                    )
                nc.sync.dma_start(
                    ctx_out[b].rearrange("(o p) -> p o", p=128), ctx_sb
                )

    nc.compile()
    return nc


_NC_CACHE = None


def _get_nc():
    global _NC_CACHE
    if _NC_CACHE is None:
        _NC_CACHE = build_program()
    return _NC_CACHE


def kernel(encoder_outputs, decoder_outputs, W_a, U_a, V_a):
    global LAST_RESULTS
    enc = np.ascontiguousarray(np.asarray(encoder_outputs, dtype=np.float32))
    dec = np.asarray(decoder_outputs, dtype=np.float32)
    W = np.ascontiguousarray(np.asarray(W_a, dtype=np.float32))
    U = np.ascontiguousarray(np.asarray(U_a, dtype=np.float32))
    V = np.ascontiguousarray(np.asarray(V_a, dtype=np.float32))

    nc = _get_nc()

    in_maps = []
    for c in range(N_CORES):
        lo, hi = c * BPC, (c + 1) * BPC
        in_maps.append(
            {
                "enc": enc[lo:hi],
                "decT": np.ascontiguousarray(dec[lo:hi, -1, :].T),
                "W": W,
                "U": U,
                "V": V,
            }
        )

    res = run_bass_kernel_spmd(
        nc, in_maps, core_ids=list(range(N_CORES)), trace=TRACE
    )
    LAST_RESULTS = res

    context = np.concatenate([r["ctx_out"] for r in res.results], axis=0)
    weights = np.concatenate([r["w_out"] for r in res.results], axis=0)
    return context, weights[..., None]


# revision 9
# speedup vs baseline: 1.2206x; 1.2206x over previous
"""Bahdanau additive attention (nn_AttentionLayer_72353019068602) on 8 trn2 cores.

Sharding: data-parallel over batch (32 -> 4 per core); W_a/U_a/V_a replicated.

Per-core pipeline (4 batches, software-pipelined so PE never idles):
  prefetch(b): enc[b] HBM->SBUF, cast bf16 (DVE), transpose (XBAR DMA)
  q[f,b]   = U_a.T @ dec_last.T          (PE, fp32, once)
  S[f,t]   = W_a.T @ encT                (PE, bf16, fp32 PSUM accum)
  th[f,t]  = tanh(S + q[f,b])            (ACT, per-partition bias)
  sc[1,t]  = V.T @ th                    (PE, M=1)
  softmax over t=2048                    (DVE+ACT on one partition)
  ctx[e]   = sum_t w[t]*encT[e,t]        (DVE mult+reduce, bf16)

prefetch(b+1) is issued BEFORE pass1(b) so each engine's FIFO has next-batch
work queued ahead of this batch's softmax/context chain -> no PE gap at batch
boundaries (HAM stays at K=8/8).
"""

import numpy as np

import concourse.bass as bass
import concourse.mybir as mybir
import concourse.tile as tile
from concourse import bacc
from concourse.bass_utils import run_bass_kernel_spmd

B, TE, TD, D = 32, 2048, 64, 1024
N_CORES = 8
BPC = B // N_CORES  # batches per core

F32 = mybir.dt.float32
BF16 = mybir.dt.bfloat16

# test.py can flip this to get an NTFF profile / exec_time_ns out of the run.
TRACE = False
LAST_RESULTS = None  # BassKernelResults of the most recent kernel() call


def build_program():
    nc = bacc.Bacc("TRN2", target_bir_lowering=False, debug=False)

    enc = nc.dram_tensor("enc", [BPC, TE, D], F32, kind="ExternalInput").ap()
    decT = nc.dram_tensor("decT", [D, BPC], F32, kind="ExternalInput").ap()
    W = nc.dram_tensor("W", [D, D], F32, kind="ExternalInput").ap()
    U = nc.dram_tensor("U", [D, D], F32, kind="ExternalInput").ap()
    V = nc.dram_tensor("V", [D, 1], F32, kind="ExternalInput").ap()
    ctx_out = nc.dram_tensor("ctx_out", [BPC, D], F32, kind="ExternalOutput").ap()
    w_out = nc.dram_tensor("w_out", [BPC, TE], F32, kind="ExternalOutput").ap()

    EC = D // 128  # 8 e (and f) chunks
    NS = TE // 128  # 16 natural subtiles per batch
    NQ = TE // 512  # 4 t-chunks of 512 per batch
    Tanh = mybir.ActivationFunctionType.Tanh
    Exp = mybir.ActivationFunctionType.Exp

    with tile.TileContext(nc) as tc:
        with (
            tc.tile_pool(name="const", bufs=1) as const_pool,
            tc.tile_pool(name="nat", bufs=4) as nat_pool,
            tc.tile_pool(name="natb", bufs=4) as natb_pool,
            tc.tile_pool(name="encT", bufs=2) as encT_pool,
            tc.tile_pool(name="th", bufs=4) as th_pool,
            tc.tile_pool(name="sm", bufs=2) as sm_pool,
            tc.tile_pool(name="wrep", bufs=2) as wrep_pool,
            tc.tile_pool(name="junk", bufs=2) as junk_pool,
            tc.tile_pool(name="mmps", bufs=4, space="PSUM") as mm_psum,
            tc.tile_pool(name="scps", bufs=2, space="PSUM") as sc_psum,
            tc.tile_pool(name="qps", bufs=2, space="PSUM") as q_psum,
            tc.tile_pool(name="dram", bufs=2, space="DRAM") as dram_pool,
        ):
            encT_tiles = {}

            def prefetch(b):
                # HBM fp32 -> SBUF -> bf16 -> DRAM bounce -> one big XBAR
                # transpose back (DRAM-source transposes run ~10x faster than
                # SBUF-source ones, which go in 256B packets).
                encbf = dram_pool.tile([TE, D], BF16, tag="encbf", name="encbf")
                for s in range(NS):
                    nat = nat_pool.tile([128, D], F32, tag="nat", name="nat")
                    nc.sync.dma_start(nat, enc[b, s * 128 : (s + 1) * 128, :])
                    natb = natb_pool.tile([128, D], BF16, tag="natb", name="natb")
                    nc.vector.tensor_copy(natb, nat)
                    nc.sync.dma_start(encbf[s * 128 : (s + 1) * 128, :], natb)
                # encT[p, o, t]: e = o*128+p
                encT = encT_pool.tile([128, EC, TE], BF16, tag="encT", name="encT")
                nc.sync.dma_start_transpose(encT, encbf)
                encT_tiles[b] = encT

            # batch 0 prefetch goes first so its transposes clear the queues
            # while the weight preload + q matmuls run.
            prefetch(0)

            # ---- preload: W (cast bf16), V (cast bf16), dec_last.T, q = U.T @ decT
            W_bf = const_pool.tile([128, EC, D], BF16)
            for ec in range(EC):
                wt = nat_pool.tile([128, D], F32, tag="nat", name="wt")
                nc.sync.dma_start(wt, W[ec * 128 : (ec + 1) * 128, :])
                nc.vector.tensor_copy(W_bf[:, ec, :], wt)

            vt = const_pool.tile([128, EC], F32)
            nc.sync.dma_start(vt, V[:, 0].rearrange("(o p) -> p o", p=128))
            V_bf = const_pool.tile([128, EC], BF16)
            nc.vector.tensor_copy(V_bf, vt)

            dec_sb = const_pool.tile([128, EC, BPC], F32)
            nc.sync.dma_start(dec_sb, decT.rearrange("(o p) b -> p o b", p=128))

            q_sb = const_pool.tile([128, EC, BPC], F32)
            for fc in range(EC):
                qp = q_psum.tile([128, BPC], F32, tag="qp", name="qp")
                for dc in range(EC):
                    ut = nat_pool.tile([128, 128], F32, tag="ut", name="ut")
                    nc.sync.dma_start(
                        ut, U[dc * 128 : (dc + 1) * 128, fc * 128 : (fc + 1) * 128]
                    )
                    nc.tensor.matmul(
                        qp,
                        lhsT=ut,
                        rhs=dec_sb[:, dc, :],
                        start=(dc == 0),
                        stop=(dc == EC - 1),
                    )
                nc.scalar.copy(q_sb[:, fc, :], qp)

            # ---- main loop over batches
            for b in range(BPC):
                if b + 1 < BPC:
                    prefetch(b + 1)
                encT = encT_tiles.pop(b)

                scores = sm_pool.tile([1, TE], F32, tag="scores", name="scores")
                for tq in range(NQ):
                    scp = sc_psum.tile([1, 512], F32, tag="scp", name="scp")
                    for fc in range(EC):
                        ps = mm_psum.tile([128, 512], F32, tag="ps", name="ps")
                        for ec in range(EC):
                            nc.tensor.matmul(
                                ps,
                                lhsT=W_bf[:, ec, fc * 128 : (fc + 1) * 128],
                                rhs=encT[:, ec, tq * 512 : (tq + 1) * 512],
                                start=(ec == 0),
                                stop=(ec == EC - 1),
                            )
                        th = th_pool.tile([128, 512], BF16, tag="th", name="th")
                        nc.scalar.activation(th, ps, Tanh, bias=q_sb[:, fc, b : b + 1])
                        nc.tensor.matmul(
                            scp,
                            lhsT=V_bf[:, fc : fc + 1],
                            rhs=th,
                            start=(fc == 0),
                            stop=(fc == EC - 1),
                        )
                    nc.vector.tensor_copy(scores[:, tq * 512 : (tq + 1) * 512], scp)

                # softmax over the 2048 scores (single partition, in place)
                negmx = sm_pool.tile([1, 1], F32, tag="negmx", name="negmx")
                nc.vector.tensor_reduce(
                    negmx, scores, axis=mybir.AxisListType.X,
                    op=mybir.AluOpType.max, negate=True,
                )
                ssum = sm_pool.tile([1, 1], F32, tag="ssum", name="ssum")
                nc.scalar.activation(scores, scores, Exp, bias=negmx, accum_out=ssum)
                rec = sm_pool.tile([1, 1], F32, tag="rec", name="rec")
                nc.vector.reciprocal(rec, ssum)
                nc.vector.tensor_scalar_mul(scores, scores, rec)
                nc.sync.dma_start(w_out[b], scores)

                # context: ctx[e] = sum_t w[t] * encT[e, t]  (bf16 on DVE)
                scores_bf = sm_pool.tile([1, TE], BF16, tag="scores_bf", name="scores_bf")
                nc.vector.tensor_copy(scores_bf, scores)
                w_rep = wrep_pool.tile([128, TE], BF16, tag="wrep", name="w_rep")
                nc.gpsimd.partition_broadcast(w_rep, scores_bf)
                ctx_sb = sm_pool.tile([128, EC], F32, tag="ctxsb", name="ctx_sb")
                for ec in range(EC):
                    junk = junk_pool.tile([128, TE], BF16, tag="junk", name="junk")
                    nc.vector.tensor_tensor(
                        junk,
                        encT[:, ec, :],
                        w_rep,
                        mybir.AluOpType.mult,
                    )
                    nc.vector.tensor_reduce(
                        ctx_sb[:, ec : ec + 1], junk,
                        axis=mybir.AxisListType.X, op=mybir.AluOpType.add,
                    )
                nc.sync.dma_start(
                    ctx_out[b].rearrange("(o p) -> p o", p=128), ctx_sb
                )

    nc.compile()
    return nc


_NC_CACHE = None


def _get_nc():
    global _NC_CACHE
    if _NC_CACHE is None:
        _NC_CACHE = build_program()
    return _NC_CACHE


def kernel(encoder_outputs, decoder_outputs, W_a, U_a, V_a):
    global LAST_RESULTS
    enc = np.ascontiguousarray(np.asarray(encoder_outputs, dtype=np.float32))
    dec = np.asarray(decoder_outputs, dtype=np.float32)
    W = np.ascontiguousarray(np.asarray(W_a, dtype=np.float32))
    U = np.ascontiguousarray(np.asarray(U_a, dtype=np.float32))
    V = np.ascontiguousarray(np.asarray(V_a, dtype=np.float32))

    nc = _get_nc()

    in_maps = []
    for c in range(N_CORES):
        lo, hi = c * BPC, (c + 1) * BPC
        in_maps.append(
            {
                "enc": enc[lo:hi],
                "decT": np.ascontiguousarray(dec[lo:hi, -1, :].T),
                "W": W,
                "U": U,
                "V": V,
            }
        )

    res = run_bass_kernel_spmd(
        nc, in_maps, core_ids=list(range(N_CORES)), trace=TRACE
    )
    LAST_RESULTS = res

    context = np.concatenate([r["ctx_out"] for r in res.results], axis=0)
    weights = np.concatenate([r["w_out"] for r in res.results], axis=0)
    return context, weights[..., None]
